# revision 1
# baseline (speedup 1.0000x reference)
"""Multi-head attention (b=4, n=2048, d=1024, h=16, dh=64) on 8 TRN2 NeuronCores.

Sharding: batch x sequence-half per core (core c handles batch b=c//2, query
rows s=(c%2)*1024 .. +1024). Each core recomputes K/V for its whole batch
locally (no collectives), computes flash-style attention for its 1024 query
rows over all 16 heads, applies the output projection, and writes a disjoint
1024-row slice of the flattened output. Matmuls run in fp32r (TF32-like fast
fp32 mode); the attention AV product runs in bf16.

Host-side layout choices (free transposes/permutes in numpy):
  xtkv [d, 2048] = concat(x[b, my_half].T, x[b, other_half].T) -- the core's
      own query rows are ALWAYS columns 0:1024, so the same SPMD graph works
      on every core, and key order permutation is softmax-invariant.
  wqt/wkt/wvt/wot = W.T (contraction dim first), bo as [1, d].
"""

import sys

sys.path.insert(0, "/opt/trn_rl_repo")

from contextlib import ExitStack

import numpy as np

import concourse.bass as bass
import concourse.tile as tile
from concourse import bacc, mybir
from concourse.bass_utils import run_bass_kernel_spmd

F32 = mybir.dt.float32
F32R = mybir.dt.float32r
BF16 = mybir.dt.bfloat16
EXP = mybir.ActivationFunctionType.Exp

P = 128
D = 1024  # model dim
NI = 1024  # query rows per core
NJ = 2048  # key rows per core (full batch)
H = 16  # heads
DH = 64  # head dim
SCALE = DH**-0.5  # 0.125
NCORES = 8

NCC = D // P  # 8 contraction chunks
NDB = D // P  # 8 feature blocks


def _build():
    nc = bacc.Bacc("TRN2", target_bir_lowering=False, debug=False, num_devices=NCORES)

    xtkv = nc.dram_tensor("xtkv", [D, NJ], BF16, kind="ExternalInput").ap()
    wqt = nc.dram_tensor("wqt", [D, D], BF16, kind="ExternalInput").ap()
    wkt = nc.dram_tensor("wkt", [D, D], BF16, kind="ExternalInput").ap()
    wvt = nc.dram_tensor("wvt", [D, D], BF16, kind="ExternalInput").ap()
    wot = nc.dram_tensor("wot", [D, D], BF16, kind="ExternalInput").ap()
    bo = nc.dram_tensor("bo", [1, D], F32, kind="ExternalInput").ap()
    out = nc.dram_tensor("out", [NI, D], BF16, kind="ExternalOutput").ap()
    DEBUG = bool(__import__("os").environ.get("KERNEL_DEBUG"))
    if DEBUG:
        dbg_kt0 = nc.dram_tensor("dbg_kt0", [P, NJ], BF16, kind="ExternalOutput").ap()
        dbg_qt0 = nc.dram_tensor("dbg_qt0", [P, NI], BF16, kind="ExternalOutput").ap()
        dbg_v0 = nc.dram_tensor("dbg_v0", [P, H, DH + 1], BF16, kind="ExternalOutput").ap()
        dbg_ctx0 = nc.dram_tensor("dbg_ctx0", [P, NI], BF16, kind="ExternalOutput").ap()

    with tile.TileContext(nc) as tc, ExitStack() as octx:
        # kernel-wide PSUM pools: 4 + 2 + 2 = 8 banks
        psA = octx.enter_context(tc.tile_pool(name="psA", bufs=2, space="PSUM"))
        psB = octx.enter_context(tc.tile_pool(name="psB", bufs=2, space="PSUM"))
        psC = octx.enter_context(tc.tile_pool(name="psC", bufs=2, space="PSUM"))
        dramp = octx.enter_context(tc.tile_pool(name="dramp", bufs=4, space="DRAM"))

        kt_pool = octx.enter_context(tc.tile_pool(name="ktp", bufs=1))
        qt_pool = octx.enter_context(tc.tile_pool(name="qtp", bufs=1))
        v_pool = octx.enter_context(tc.tile_pool(name="vp", bufs=1))
        KT = [kt_pool.tile([P, NJ], BF16, tag=f"kt{i}", name=f"kt{i}") for i in range(NDB)]
        QT = [qt_pool.tile([P, NI], BF16, tag=f"qt{i}", name=f"qt{i}") for i in range(NDB)]
        vall = v_pool.tile([P, NJ // P, H, DH + 1], BF16, tag="vall", name="vall")
        V = [vall[:, j] for j in range(NJ // P)]

        # attention pools first: their SBUF must not alias the projection pools
        ctx_pool = octx.enter_context(tc.tile_pool(name="ctxp", bufs=1, side="right"))
        CTX = [ctx_pool.tile([P, NI], BF16, tag=f"ctx{t}", name=f"ctx{t}") for t in range(NDB)]
        esp = octx.enter_context(tc.tile_pool(name="es", bufs=11))
        recp = octx.enter_context(tc.tile_pool(name="rec", bufs=6))
        stp = octx.enter_context(tc.tile_pool(name="stg", bufs=10))

        # ---------------- phase Q (bf16); XQ reused by K; WK/XKB prefetched ----
        xqp = octx.enter_context(tc.tile_pool(name="xq", bufs=1))
        XQ = [xqp.tile([P, NI], BF16, tag=f"xq{c}", name=f"xq{c}") for c in range(NCC)]
        wkp = octx.enter_context(tc.tile_pool(name="wk", bufs=1))
        WK = [wkp.tile([P, D], BF16, tag=f"wk{c}", name=f"wk{c}") for c in range(NCC)]
        with tc.tile_pool(name="wq", bufs=1) as wqp:
            WQ = [wqp.tile([P, D], BF16, tag=f"wq{c}", name=f"wq{c}") for c in range(NCC)]
            for c in range(NCC):
                nc.sync.dma_start(XQ[c][:], xtkv[c * P : (c + 1) * P, 0:NI])
                nc.sync.dma_start(WQ[c][:], wqt[c * P : (c + 1) * P, :])
            for c in range(NCC):
                nc.sync.dma_start(WK[c][:], wkt[c * P : (c + 1) * P, :])
            for db in range(NDB):
                for ib in range(NI // 512):
                    ps = psB.tile([P, 512], F32, tag="pj", name="pj")
                    for c in range(NCC):
                        nc.tensor.matmul(
                            ps[:],
                            WQ[c][:, db * P : (db + 1) * P],
                            XQ[c][:, ib * 512 : (ib + 1) * 512],
                            start=(c == 0),
                            stop=(c == NCC - 1),
                        )
                    nc.vector.tensor_copy(QT[db][:, ib * 512 : (ib + 1) * 512], ps[:])

        # Wo pools open before the fused scope so WO/bias prefetch during it
        wop = octx.enter_context(tc.tile_pool(name="wo", bufs=1))
        bip = octx.enter_context(tc.tile_pool(name="bias", bufs=1))
        osp = octx.enter_context(tc.tile_pool(name="os", bufs=6))
        WO = [wop.tile([P, D], BF16, tag=f"wo{f}", name=f"wo{f}") for f in range(NCC)]
        for f in range(NCC):
            nc.sync.dma_start(WO[f][:], wot[f * P : (f + 1) * P, :])
        BIAS = bip.tile([P, D], F32, name="BIAS")
        nc.gpsimd.dma_start(BIAS[:], bo.to_broadcast([P, D]))

        # -------- fused phase K + attention: per db, project KT[db] then emit
        # the two heads (x2 ib blocks) that consume it ------------------------
        with (
            tc.tile_pool(name="xkb", bufs=1) as xkbp,
            tc.tile_pool(name="wvh", bufs=1) as wvhp,
        ):
            XKB = [xkbp.tile([P, NI], BF16, tag=f"xkb{c}", name=f"xkb{c}") for c in range(NCC)]
            XKA = XQ
            for c in range(NCC):
                nc.sync.dma_start(XKB[c][:], xtkv[c * P : (c + 1) * P, NI:NJ])
            for j in range(NJ // P):
                nc.vector.memset(V[j][:, :, DH : DH + 1], 1.0)

            def v_halfpass_jg(vh, jg):
                """Project V head-half vh for key group jg (4 j-blocks); x is
                sliced straight out of the resident XKA/XKB tiles."""
                for j4 in range(4):
                    j = jg * 4 + j4
                    xh = XKA if j < 8 else XKB
                    jloc = j % 8
                    ps = psB.tile([P, 512], F32, tag="pj", name="pj")
                    for c in range(NCC):
                        nc.tensor.matmul(
                            ps[:],
                            xh[c][:, jloc * P : (jloc + 1) * P],
                            WVH[c][:],
                            start=(c == 0),
                            stop=(c == NCC - 1),
                        )
                    nc.vector.tensor_copy(
                        V[j][:, vh * 8 : (vh + 1) * 8, 0:DH],
                        ps[:].rearrange("p (h d) -> p h d", h=8),
                    )

            WVH = [wvhp.tile([P, 512], BF16, tag=f"wvh{c}", name=f"wvh{c}") for c in range(NCC)]
            # V head-half 0 (heads 0-7): needed from db=0
            for c in range(NCC):
                nc.sync.dma_start(WVH[c][:], wvt[c * P : (c + 1) * P, 0:512])
            for jg in range(NJ // 512):
                v_halfpass_jg(0, jg)

            for db in range(NDB):
                # V head-half 1 (heads 8-15): one key group per db in 1..4
                if db == 1:
                    WVH = [
                        wvhp.tile([P, 512], BF16, tag=f"wvh{c}", name=f"wvh{c}2")
                        for c in range(NCC)
                    ]
                    for c in range(NCC):
                        nc.sync.dma_start(WVH[c][:], wvt[c * P : (c + 1) * P, 512:1024])
                if 1 <= db <= 4:
                    v_halfpass_jg(1, db - 1)
                # K projection for this db
                for jb in range(NJ // 512):
                    half = XKA if jb < 2 else XKB
                    cslc = slice((jb % 2) * 512, (jb % 2) * 512 + 512)
                    ps = psB.tile([P, 512], F32, tag="pj", name="pj")
                    for c in range(NCC):
                        nc.tensor.matmul(
                            ps[:],
                            WK[c][:, db * P : (db + 1) * P],
                            half[c][:, cslc],
                            start=(c == 0),
                            stop=(c == NCC - 1),
                        )
                    nc.vector.tensor_copy(KT[db][:, jb * 512 : (jb + 1) * 512], ps[:])
                # the two heads living in KT[db], for both i-blocks
                t = db
                for ib in range(NI // 512):
                    islc = slice(ib * 512, (ib + 1) * 512)
                    stgs = {}
                    for hh in range(2):
                        h = 2 * db + hh
                        dp = hh * DH
                        es_list = []
                        for pr in range(NJ // 256):
                            sp = psA.tile([P, 1024], F32, tag="sp", name="sp")
                            for half2 in range(2):
                                j = pr * 2 + half2
                                nc.tensor.matmul(
                                    sp[:, half2 * 512 : (half2 + 1) * 512],
                                    KT[t][dp : dp + DH, j * P : (j + 1) * P],
                                    QT[t][dp : dp + DH, islc],
                                    start=True,
                                    stop=True,
                                )
                            es = esp.tile([P, 1024], BF16, tag="es", name="es")
                            nc.scalar.activation(es[:], sp[:], EXP, scale=SCALE)
                            es_list.append(es)
                        # AV with es STATIONARY and V|1 MOVING: N=65 cycles per
                        # matmul instead of 512 -- output [i, d|sum] per i-128
                        # block, so the softmax divide is a native per-partition
                        # tensor_scalar, and a bf16 DMA-transpose restores the
                        # [f, i] layout Wo consumes.
                        for q in range(4):
                            ctp = psC.tile([P, DH + 1], F32, tag="ct", name="ct")
                            for j in range(NJ // P):
                                nc.tensor.matmul(
                                    ctp[:],
                                    es_list[j // 2][
                                        :,
                                        (j % 2) * 512 + q * P : (j % 2) * 512 + (q + 1) * P,
                                    ],
                                    V[j][:, h, :],
                                    start=(j == 0),
                                    stop=(j == NJ // P - 1),
                                )
                            rec = recp.tile([P, 1], F32, tag="rec", name="rec")
                            nc.vector.reciprocal(rec[:], ctp[:, DH : DH + 1])
                            if hh == 0:
                                stgs[q] = stp.tile([P, 2 * DH], BF16, tag="st", name="st")
                            stg = stgs[q]
                            nc.vector.tensor_scalar_mul(
                                stg[:, dp : dp + DH], ctp[:, 0:DH], rec[:]
                            )
                            if hh == 1:
                                # both heads of the pair staged: one 128-wide
                                # bf16 DMA-transpose fills CTX tile t's columns
                                nc.sync.dma_start_transpose(
                                    CTX[t][:, ib * 512 + q * P : ib * 512 + (q + 1) * P],
                                    stg[:],
                                )

        if DEBUG:
            nc.sync.dma_start(dbg_ctx0, CTX[0][:])

        # ---------------- phase Wo: out = CTX.T @ WoT + bo ----------------------
        if True:
            for ib8 in range(NI // P):
                for eb in range(2):
                    ps = psB.tile([P, 512], F32, tag="pj", name="pj")
                    for f in range(NCC):
                        nc.tensor.matmul(
                            ps[:],
                            CTX[f][:, ib8 * P : (ib8 + 1) * P],
                            WO[f][:, eb * 512 : (eb + 1) * 512],
                            start=(f == 0),
                            stop=(f == NCC - 1),
                        )
                    ostage = osp.tile([P, 512], BF16, tag="os", name="os")
                    nc.vector.tensor_add(
                        ostage[:], ps[:], BIAS[:, eb * 512 : (eb + 1) * 512]
                    )
                    nc.sync.dma_start(
                        out[ib8 * P : (ib8 + 1) * P, eb * 512 : (eb + 1) * 512],
                        ostage[:],
                    )

    nc.compile()
    return nc


_NC = None


def _get_nc():
    global _NC
    if _NC is None:
        _NC = _build()
    return _NC


def _make_in_maps(x, Wq, Wk, Wv, Wo, bo):
    import ml_dtypes

    bf16 = ml_dtypes.bfloat16
    wqt = np.ascontiguousarray(Wq.T).astype(bf16)
    wkt = np.ascontiguousarray(Wk.T).astype(bf16)
    wvt = np.ascontiguousarray(Wv.T).astype(bf16)
    wot = np.ascontiguousarray(Wo.T).astype(bf16)
    bo2 = np.ascontiguousarray(bo.reshape(1, D)).astype(np.float32)
    in_maps = []
    for c in range(NCORES):
        b, s = c // 2, c % 2
        mine = x[b, s * NI : (s + 1) * NI, :].T
        other = x[b, (1 - s) * NI : (2 - s) * NI, :].T
        xtkv = np.ascontiguousarray(np.concatenate([mine, other], axis=1)).astype(bf16)
        in_maps.append(
            {"xtkv": xtkv, "wqt": wqt, "wkt": wkt, "wvt": wvt, "wot": wot, "bo": bo2}
        )
    return in_maps


def _run(x, Wq, Wk, Wv, Wo, bo, **spmd_kwargs):
    nc = _get_nc()
    in_maps = _make_in_maps(x, Wq, Wk, Wv, Wo, bo)
    res = run_bass_kernel_spmd(nc, in_maps, list(range(NCORES)), **spmd_kwargs)
    outs = [np.asarray(res.results[c]["out"]) for c in range(NCORES)]
    full = np.concatenate(outs, axis=0).reshape(4, 2048, D).astype(np.float32)
    return full, res


def kernel(x, Wq, Wk, Wv, Wo, bo):
    full, _ = _run(
        np.asarray(x), np.asarray(Wq), np.asarray(Wk), np.asarray(Wv),
        np.asarray(Wo), np.asarray(bo),
    )
    return full

